# revision 11
# baseline (speedup 1.0000x reference)
"""Trainium2 Bass kernel for nn_DeformableBlock (offset-conv -> deformable
conv v1 -> GroupNorm(32) -> ReLU), 8-core SPMD.

Sharding: core c -> (batch b = c//2, row-half h = c%2), rows [32h, 32h+32).
GroupNorm statistics are AllReduce'd across each (b,0)/(b,1) core pair.

v2 pipeline (vs v1): offset conv folded into per-tile fp16 matmuls over the
xz window (zoff), with the 3x3 shifted-tap sum done via a small DRAM
round-trip; z table in fp16 produced tap-major so gathers start early;
gather buffers multi-buffered so SWDGE dispatch (Pool), DVE accumulate,
PE matmuls and HBM traffic all overlap.

Per-core algorithm (z-first formulation):
  z_k = x . W_k (pointwise matmul per 3x3 tap) over a 40-row window, stored
  fp16 in DRAM as y-pair rows ypt[j] = (z[j], z[j+64]); one dma_gather per
  tap with OVERLAPPING 2KB elements (elem_size=1024, elem_step=512) reads
  ypt rows j and j+1 per index = all four bilinear corners in one
  descriptor, then fused scalar_tensor_tensor accumulate (fp16, DVE 2x).
Gather indices are computed ON DEVICE directly in the SWDGE wrapped-16 idx
layout via 8 host-constant permutation matmuls on the (py,px) grid.
"""
import functools
import numpy as np

import concourse.bass as bass
import concourse.bacc as bacc
import concourse.mybir as mybir
import concourse.tile as tile
from concourse.bass_utils import run_bass_kernel_spmd

F32 = mybir.dt.float32
FP16 = mybir.dt.float16
I16 = mybir.dt.int16
I32 = mybir.dt.int32
AOP = mybir.AluOpType
ACT = mybir.ActivationFunctionType

B, CIN, COUT, H, W = 4, 256, 256, 64, 64
K = 9
WROWS = 40            # z window rows (w0 = r0 - 4)
NPOS = 2048           # output positions per core (32 rows)
NWIN = WROWS * 64     # z window positions (2560)
NTW = NWIN // 128     # window tiles (20)
ZPAD = 72             # guard rows before the y-pair z table
NZROW = NWIN + 144    # 2704
NT = 16               # output position tiles of 128
EPS = 1e-5
GN_N = 2 * NPOS * 8   # elements per GN group (both cores of the pair)
J0 = 4 * 64           # window index of output position 0


def build_program(use_cc=True):
    nc = bacc.Bacc(None, target_bir_lowering=False, num_devices=8)

    # ---------------- I/O ----------------
    xz_d = nc.dram_tensor("xz", [2, 128, NWIN], FP16, kind="ExternalInput")
    wdef_d = nc.dram_tensor("wdef", [2, 128, K, COUT], FP16, kind="ExternalInput")
    woffz_d = nc.dram_tensor("woffz", [2, 128, K, 18], FP16, kind="ExternalInput")
    bxy_d = nc.dram_tensor("bxy", [128, NT, 18], F32, kind="ExternalInput")
    # border masks: col 0 zeros partitions {0,64} (kx=0 taps), col 1 zeros
    # partitions {63,127} (kx=2 taps) — reference conv zero-pads x
    bmask_d = nc.dram_tensor("bmask", [128, 2], F32, kind="ExternalInput")
    # per-core scalar replicated to [128,1]: y window-row bias (-12 - r0)
    wconst_d = nc.dram_tensor("wconst", [128, 1], F32, kind="ExternalInput")
    pmat_d = nc.dram_tensor("pmat", [8, 128, 128], F32, kind="ExternalInput")
    onescol_d = nc.dram_tensor("onescol", [128, 1], F32, kind="ExternalInput")
    onesrow_d = nc.dram_tensor("onesrow", [1, 128], F32, kind="ExternalInput")
    gnab_d = nc.dram_tensor("gnab", [1, 512], F32, kind="ExternalInput")
    out_d = nc.dram_tensor("out", [NPOS, COUT], F32, kind="ExternalOutput")

    with tile.TileContext(nc) as tc:
        with (
            tc.tile_pool(name="const", bufs=1) as cpool,
            tc.tile_pool(name="wm", bufs=1) as wmpool,
            tc.tile_pool(name="zst", bufs=2) as zstpool,
            tc.tile_pool(name="g", bufs=4) as gpool,
            tc.tile_pool(name="acc", bufs=1) as accpool,
            tc.tile_pool(name="outp", bufs=2) as outpool,
            tc.tile_pool(name="sh", bufs=2) as shpool,
            tc.tile_pool(name="ps", bufs=2, space="PSUM") as pspool,
            tc.tile_pool(name="ps2", bufs=1, space="PSUM") as ps2pool,
            tc.tile_pool(name="dram", bufs=1, space="DRAM") as dpool,
        ):
            # ---------------- load constants / inputs ----------------
            xz = cpool.tile([128, 2, NWIN], FP16, tag="xz", name="xz")
            for ci in range(2):
                nc.sync.dma_start(xz[:, ci], xz_d[ci])
            wdef = cpool.tile([128, 2, K, COUT], FP16, tag="wdef", name="wdef")
            woffz = cpool.tile([128, 2, K, 18], FP16, tag="woffz", name="woffz")
            for ci in range(2):
                nc.sync.dma_start(wdef[:, ci], wdef_d[ci])
                nc.sync.dma_start(woffz[:, ci], woffz_d[ci])
            bxy = cpool.tile([128, NT, 18], F32, tag="bxy", name="bxy")
            nc.sync.dma_start(bxy[:], bxy_d[:])
            bmask = cpool.tile([128, 2], F32, tag="bmask", name="bmask")
            nc.sync.dma_start(bmask[:], bmask_d[:])
            wconst = cpool.tile([128, 1], F32, tag="wconst", name="wconst")
            nc.sync.dma_start(wconst[:], wconst_d[:])
            pmat = cpool.tile([128, 8, 128], F32, tag="pmat", name="pmat")
            nc.sync.dma_start(pmat[:], pmat_d[:].rearrange("u p m -> p u m"))
            onescol = cpool.tile([128, 1], F32, tag="onescol", name="onescol")
            nc.sync.dma_start(onescol[:], onescol_d[:])
            onesrow = cpool.tile([1, 128], F32, tag="onesrow", name="onesrow")
            nc.sync.dma_start(onesrow[:], onesrow_d[:])
            gnab = cpool.tile([1, 512], F32, tag="gnab", name="gnab")
            nc.sync.dma_start(gnab[:], gnab_d[:])

            zbufs = [dpool.tile([NZROW, 2 * COUT], FP16, tag=f"zbuf{k}",
                                name=f"zbuf{k}") for k in range(K)]
            zoffb = dpool.tile([NWIN, K * 18], F32, tag="zoffb", name="zoffb")
            ccin = dpool.tile([1, 64], F32, tag="ccin", name="ccin")
            ccout = dpool.tile([1, 64], F32, tag="ccout", name="ccout")

            # zero guard/boundary rows of every tap's zquad table; stores
            # overwrite the live slots afterwards; stale quad slots stay 0.
            zguard = cpool.tile([128, 2 * COUT], FP16, tag="zg", name="zg")
            nc.vector.memset(zguard[:], 0)
            gb0 = ZPAD + NWIN - 64    # 2568: first row with no s=1 write

            for k in range(K):
                zb = zbufs[k][:]
                for i, (base, nrow) in enumerate(((60, 12), (gb0, 80))):
                    wr = bass.AP(
                        zb.tensor, zb.offset + base * 2 * COUT,
                        [[2 * COUT, nrow], [1, 2 * COUT]])
                    (nc.sync if i == 0 else nc.scalar).dma_start(
                        wr, zguard[0:nrow, :])

            # ---------------- zoff: per-window-tile offset contributions ----
            # zoff[j, k, :] = sum_cin x[cin, j] * woff[cin, k, :]
            woffz_f = [woffz[:, ci].rearrange("p k c -> p (k c)")
                       for ci in range(2)]
            for tt in range(1, 19):  # window tiles covering J0 +/- 65
                zops = ps2pool.tile([128, K * 18], F32, bufs=2, tag="zops",
                                    name="zops")
                nc.tensor.matmul(zops[:], xz[:, 0, 128 * tt:128 * (tt + 1)],
                                 woffz_f[0], start=True, stop=False)
                nc.tensor.matmul(zops[:], xz[:, 1, 128 * tt:128 * (tt + 1)],
                                 woffz_f[1], start=False, stop=True)
                zost = shpool.tile([128, K * 18], F32, tag="zost", name="zost")
                nc.scalar.copy(zost[:], zops[:])
                nc.sync.dma_start(zoffb[128 * tt:128 * (tt + 1), :], zost[:])

            # ---------------- offsets: 9 shifted loads + masked sum --------
            # off(p) = sum_k zoff[J0 + p + s_k, k, :], s_k = 64*(ky-1)+(kx-1)
            pxy = cpool.tile([128, NT, 18], F32, tag="pxy", name="pxy")
            zo = zoffb[:]
            for k in range(K):
                ky, kx = k // 3, k % 3
                sk = 64 * (ky - 1) + (kx - 1)
                sh = shpool.tile([128, NT, 18], F32, tag="sh", name=f"sh{k}")
                src = bass.AP(
                    zo.tensor, zo.offset + (J0 + sk) * K * 18 + 18 * k,
                    [[K * 18, 128], [128 * K * 18, NT], [1, 18]])
                nc.scalar.dma_start(sh[:], src)
                base = bxy[:] if k == 0 else pxy[:]
                if kx == 1:
                    nc.vector.tensor_add(pxy[:], base, sh[:])
                else:
                    # border columns: reference conv zero-pads x
                    nc.vector.scalar_tensor_tensor(
                        pxy[:], sh[:], bmask[:, kx // 2:kx // 2 + 1], base,
                        op0=AOP.mult, op1=AOP.add)

            # ---------------- bilinear weights (plain layout, DVE) ------
            py_sl = pxy[:, :, 0:18:2]
            px_sl = pxy[:, :, 1:18:2]

            def wm(tag):
                return wmpool.tile([128, NT, K], F32, tag=tag, name=tag)

            def dev_floor(src, tag, shape=None):
                pool_shape = shape or [128, NT, K]
                big = "b" if shape else "s"
                ii = wmpool.tile(pool_shape, I32, tag="flr_i" + big,
                                 name=tag + "i")
                ff = wmpool.tile(pool_shape, F32, tag=tag + "f", name=tag + "f")
                gt = wmpool.tile(pool_shape, F32, tag="flr_g" + big,
                                 name=tag + "g")
                nc.vector.tensor_copy(ii[:], src)        # fp32 -> int32
                nc.vector.tensor_copy(ff[:], ii[:])      # int32 -> fp32
                nc.vector.tensor_tensor(gt[:], ff[:], src, op=AOP.is_gt)
                nc.vector.tensor_tensor(ff[:], ff[:], gt[:], op=AOP.subtract)
                return ff

            y0 = dev_floor(py_sl, "y0")
            x0 = dev_floor(px_sl, "x0")
            ty = wm("ty"); tx = wm("tx")
            nc.vector.tensor_tensor(ty[:], py_sl, y0[:], op=AOP.subtract)
            nc.vector.tensor_tensor(tx[:], px_sl, x0[:], op=AOP.subtract)
            y1 = wm("y1"); x1 = wm("x1")
            nc.vector.tensor_scalar_add(y1[:], y0[:], 1.0)
            nc.vector.tensor_scalar_add(x1[:], x0[:], 1.0)

            # validity from the lifted bounds [16, 79]
            vys = []
            for (yy, vtag) in ((y0, "0"), (y1, "1")):
                yg = wm("yg"); vy = wm("vy" + vtag)
                nc.vector.tensor_scalar(yg[:], yy[:], 16.0, 79.0,
                                        op0=AOP.max, op1=AOP.min)
                nc.vector.tensor_tensor(vy[:], yg[:], yy[:], op=AOP.is_equal)
                vys.append(vy)
            vxs = []
            for (xx, vtag) in ((x0, "0"), (x1, "1")):
                xg = wm("yg"); vx = wm("vx" + vtag)
                nc.vector.tensor_scalar(xg[:], xx[:], 16.0, 79.0,
                                        op0=AOP.max, op1=AOP.min)
                nc.vector.tensor_tensor(vx[:], xg[:], xx[:], op=AOP.is_equal)
                vxs.append(vx)

            omty = wm("omty"); omtx = wm("omtx")
            nc.vector.tensor_scalar(omty[:], ty[:], -1.0, 1.0,
                                    op0=AOP.mult, op1=AOP.add)
            nc.vector.tensor_scalar(omtx[:], tx[:], -1.0, 1.0,
                                    op0=AOP.mult, op1=AOP.add)
            wy = []
            for i, frac in enumerate((omty, ty)):
                wv = wm("wy" + str(i))
                nc.vector.tensor_tensor(wv[:], frac[:], vys[i][:], op=AOP.mult)
                wy.append(wv)
            wx = []
            for i, frac in enumerate((omtx, tx)):
                wv = wm("wx" + str(i))
                nc.vector.tensor_tensor(wv[:], frac[:], vxs[i][:], op=AOP.mult)
                wx.append(wv)

            # corner weights, laid out [128, kj, t] (kj = k*4 + 2*jy + jx)
            wgt_t = cpool.tile([128, 36, NT], F32, tag="wgt", name="wgt")
            for jy in range(2):
                for jx in range(2):
                    j = 2 * jy + jx
                    nc.vector.tensor_tensor(
                        wgt_t[:, j:36:4, :].rearrange("p k t -> p t k"),
                        wy[jy][:], wx[jx][:], op=AOP.mult)
            wgt16 = cpool.tile([128, 36, NT], FP16, tag="wgt16", name="wgt16")
            nc.vector.tensor_copy(
                wgt16[:].rearrange("p k t -> p (k t)"),
                wgt_t[:].rearrange("p k t -> p (k t)"))

            # ---------------- z matmuls for tap 0 (fills PE wait window) --
            def z_tap(k):
                zst = zstpool.tile([128, NTW, COUT], FP16, tag="zst",
                                   name=f"zst{k}")
                for tt in range(NTW):
                    zps = pspool.tile([128, COUT], F32, tag="zps", name="zps")
                    nc.tensor.matmul(zps[:], xz[:, 0, 128 * tt:128 * (tt + 1)],
                                     wdef[:, 0, k, :], start=True, stop=False)
                    nc.tensor.matmul(zps[:], xz[:, 1, 128 * tt:128 * (tt + 1)],
                                     wdef[:, 1, k, :], start=False, stop=True)
                    nc.scalar.copy(zst[:, tt, :], zps[:])
                zb = zbufs[k][:]
                engs = (nc.sync, nc.scalar)
                for s, dlt in enumerate((0, 64)):
                    wrS = bass.AP(
                        zb.tensor,
                        zb.offset + (ZPAD - dlt) * 2 * COUT + s * COUT,
                        [[2 * COUT, 128], [128 * 2 * COUT, NTW], [1, COUT]])
                    engs[s].dma_start(wrS, zst[:])

            z_tap(0)

            # ---------------- replicate (py,px) into wrapped layout -------
            # pxyr[16a+v, s, :] = pxy(p = 16s+v) via permutation matmuls P_u
            pxyr = cpool.tile([128, 128, 18], F32, tag="pxyr", name="pxyr")
            for u in range(8):
                rps = ps2pool.tile([128, NT, 18], F32, bufs=2, tag="rps",
                                   name="rps")
                nc.tensor.matmul(
                    rps[:].rearrange("p t c -> p (t c)"), pmat[:, u, :],
                    pxy[:].rearrange("p t c -> p (t c)"),
                    start=True, stop=True)
                nc.vector.tensor_copy(pxyr[:, u:128:8, :], rps[:])

            # ---------------- gather indices (replicated layout) ---------
            pyr_sl = pxyr[:, :, 0:18:2].rearrange("p s k -> p k s")
            pxr_sl = pxyr[:, :, 1:18:2].rearrange("p s k -> p k s")
            y0r = dev_floor(pyr_sl, "y0r", shape=[128, K, 128])
            x0r = dev_floor(pxr_sl, "x0r", shape=[128, K, 128])
            rwp = wmpool.tile([128, K, 128], F32, tag="flr_ib", name="rwp")
            nc.vector.tensor_scalar_add(rwp[:], y0r[:], wconst[:, 0:1])
            rw0 = wmpool.tile([128, K, 128], F32, tag="flr_gb", name="rw0")
            nc.vector.tensor_scalar(rw0[:], rwp[:], 0.0, float(WROWS - 1),
                                    op0=AOP.max, op1=AOP.min)
            # idx = rw*64 + (x0r - 16) + ZPAD  (ypair row units)
            idxf = wmpool.tile([128, K, 128], F32, tag="y0rf", name="idxf")
            nc.vector.tensor_scalar(
                rw0[:], rw0[:], 64.0, float(ZPAD - 16),
                op0=AOP.mult, op1=AOP.add)
            nc.vector.tensor_tensor(idxf[:], rw0[:], x0r[:], op=AOP.add)
            idx16 = wmpool.tile([128, K, 128], I16, tag="idx16", name="idx16")
            nc.vector.tensor_copy(
                idx16[:].rearrange("p k s -> p (k s)"),
                idxf[:].rearrange("p k s -> p (k s)"))

            # ---------------- z matmuls + store fp16 y-pair rows ----------
            for k in range(1, K):
                z_tap(k)

            # ---------------- gather + weighted accumulate ----------------
            acc = accpool.tile([128, NT, COUT], FP16, tag="acc", name="acc")
            psums = wmpool.tile([128, NT, 32], F32, tag="psums", name="psums")
            psqs = wmpool.tile([128, NT, 32], F32, tag="psqs", name="psqs")
            AX = mybir.AxisListType.X
            for k in range(K):
                zb = zbufs[k][:]
                zk = bass.AP(zb.tensor, zb.offset,
                             [[2 * COUT, NZROW - 1], [1, 4 * COUT]])
                for hh in range(2):  # num_idxs>1024 overflows SWDGE ring
                    g = gpool.tile([128, 8, 4 * COUT], FP16, tag="g",
                                   name=f"g{k}_{hh}")
                    nc.gpsimd.dma_gather(
                        out_ap=g[:],
                        in_ap=zk,
                        idxs_ap=idx16[:, k, 64 * hh:64 * (hh + 1)],
                        num_idxs=NPOS // 2,
                        num_idxs_reg=NPOS // 2,
                        elem_size=4 * COUT,
                        elem_step=2 * COUT,
                    )
                    for t in range(8 * hh, 8 * hh + 8):
                        tg = t - 8 * hh
                        for s in range(4):
                            j = (0, 2, 1, 3)[s]
                            first = (k == 0 and s == 0)
                            nc.vector.scalar_tensor_tensor(
                                acc[:, t, :],
                                g[:, tg, s * COUT:(s + 1) * COUT],
                                wgt16[:, 4 * k + j, t:t + 1],
                                g[:, tg, 0:COUT] if first else acc[:, t, :],
                                op0=AOP.mult,
                                op1=AOP.bypass if first else AOP.add)
                        if k == K - 1:
                            # tile t is final: fold its GN stats in now
                            sqt = outpool.tile([128, COUT], FP16, tag="sqt",
                                               name="sqt")
                            nc.scalar.square(sqt[:], acc[:, t, :])
                            nc.vector.tensor_reduce(
                                psums[:, t, :],
                                acc[:, t, :].rearrange("p (g c) -> p g c", c=8),
                                axis=AX, op=AOP.add)
                            nc.vector.tensor_reduce(
                                psqs[:, t, :],
                                sqt[:].rearrange("p (g c) -> p g c", c=8),
                                axis=AX, op=AOP.add)

            # ---------------- GroupNorm stats + AllReduce ----------------
            stats = wmpool.tile([128, 64], F32, tag="stats", name="stats")
            nc.vector.tensor_reduce(
                stats[:, 0:32], psums[:].rearrange("p t g -> p g t"),
                axis=AX, op=AOP.add)
            nc.vector.tensor_reduce(
                stats[:, 32:64], psqs[:].rearrange("p t g -> p g t"),
                axis=AX, op=AOP.add)
            sps = ps2pool.tile([1, 64], F32, tag="sps", name="sps")
            nc.tensor.matmul(sps[:], onescol[:], stats[:], start=True, stop=True)
            stat_row = wmpool.tile([1, 64], F32, tag="strow", name="strow")
            nc.vector.tensor_copy(stat_row[:], sps[:])
            nc.sync.dma_start(ccin[:], stat_row[:])
            if use_cc:
                nc.gpsimd.collective_compute(
                    "AllReduce", AOP.add,
                    replica_groups=[[0, 1], [2, 3], [4, 5], [6, 7]],
                    ins=[ccin[:].opt()], outs=[ccout[:].opt()],
                )
            else:
                nc.sync.dma_start(ccout[:], ccin[:])
            allst = wmpool.tile([1, 64], F32, tag="allst", name="allst")
            nc.sync.dma_start(allst[:], ccout[:])

            # mu = S/n; var = Q/n - mu^2; A = gamma*rstd; B = beta - mu*A
            mu = wmpool.tile([1, 32], F32, tag="mu", name="mu")
            var = wmpool.tile([1, 32], F32, tag="var", name="var")
            rstd = wmpool.tile([1, 32], F32, tag="rstd", name="rstd")
            nc.vector.tensor_scalar_mul(mu[:], allst[:, 0:32], 1.0 / GN_N)
            nc.vector.tensor_scalar_mul(var[:], allst[:, 32:64], 1.0 / GN_N)
            nc.vector.tensor_tensor(rstd[:], mu[:], mu[:], op=AOP.mult)
            nc.vector.tensor_tensor(var[:], var[:], rstd[:], op=AOP.subtract)
            nc.vector.tensor_scalar_add(var[:], var[:], EPS)
            nc.scalar.activation(rstd[:], var[:], ACT.Sqrt, bias=0.0)
            nc.vector.reciprocal(rstd[:], rstd[:])
            abrow = wmpool.tile([1, 512], F32, tag="abrow", name="abrow")
            rrep = wmpool.tile([1, 512], F32, tag="rrep", name="rrep")
            for c in range(8):
                nc.vector.tensor_copy(rrep[0:1, c:256:8], rstd[:])
                nc.vector.tensor_copy(rrep[0:1, 256 + c:512:8], mu[:])
            nc.vector.tensor_tensor(
                abrow[:, 0:256], rrep[:, 0:256], gnab[:, 0:256], op=AOP.mult)
            nc.vector.tensor_tensor(
                abrow[:, 256:512], rrep[:, 256:512], abrow[:, 0:256],
                op=AOP.mult)
            nc.vector.tensor_tensor(
                abrow[:, 256:512], gnab[:, 256:512], abrow[:, 256:512],
                op=AOP.subtract)
            abps = ps2pool.tile([128, 512], F32, tag="abps", name="abps")
            nc.tensor.matmul(abps[:], onesrow[:], abrow[:], start=True, stop=True)
            abbc = cpool.tile([128, 512], FP16, tag="abbc", name="abbc")
            nc.scalar.copy(abbc[:], abps[:])

            # ---------------- apply GN + ReLU, write out ----------------
            for t in range(NT):
                ot = outpool.tile([128, COUT], FP16, tag="ot", name="ot")
                nc.vector.tensor_tensor(ot[:], acc[:, t, :], abbc[:, 0:256],
                                        op=AOP.mult)
                nc.vector.tensor_tensor(ot[:], ot[:], abbc[:, 256:512],
                                        op=AOP.add)
                ot32 = outpool.tile([128, COUT], F32, tag="ot32", name="ot32")
                nc.scalar.activation(ot32[:], ot[:], ACT.Relu)
                od_ap = out_d[:, :]
                wro = bass.AP(od_ap.tensor, od_ap.offset + t * 128 * COUT,
                              [[COUT, 128], [1, COUT]])
                (nc.sync if t % 2 == 0 else nc.scalar).dma_start(wro, ot32[:])

    nc.compile()
    return nc


@functools.lru_cache(maxsize=1)
def _program():
    return build_program()


def _prep_core(core, x, offw, offb, dw):
    b, h = core // 2, core % 2
    r0 = 32 * h
    w0 = r0 - 4

    xzarr = np.zeros((2, 128, WROWS, 64), np.float32)
    for i, r in enumerate(range(w0, w0 + WROWS)):
        if 0 <= r < H:
            xzarr[0, :, i, :] = x[b, 0:128, r, :]
            xzarr[1, :, i, :] = x[b, 128:256, r, :]

    # weights: wdef[ci, c, k, o] = dw[o, ci*128+c, ky, kx]
    dwr = dw.reshape(COUT, CIN, K).transpose(1, 2, 0)     # [cin, k, o]
    wdef = np.ascontiguousarray(dwr.reshape(2, 128, K, COUT))
    owr = offw.reshape(18, CIN, K).transpose(1, 2, 0)      # [cin, k, 18]
    woffz = np.ascontiguousarray(owr.reshape(2, 128, K, 18))

    pos = np.arange(NPOS)
    prow = r0 + pos // 64
    pcol = pos % 64
    ky = np.arange(K) // 3
    kx = np.arange(K) % 3
    # lifted (+16) base grids with offset bias folded in
    by = prow[:, None] - 1.0 + ky[None, :] + offb[0::2][None, :] + 16.0
    bx = pcol[:, None] - 1.0 + kx[None, :] + offb[1::2][None, :] + 16.0
    # plain layout: [NPOS, K] -> [128, NT, K] with position p at (p%128, p//128)
    byc = by.reshape(NT, 128, K).transpose(1, 0, 2)
    bxc = bx.reshape(NT, 128, K).transpose(1, 0, 2)
    bxy = np.empty((128, NT, 18), np.float32)
    bxy[:, :, 0::2] = byc
    bxy[:, :, 1::2] = bxc

    wconst = np.full((128, 1), float(-12 - r0), np.float32)
    bmask = np.ones((128, 2), np.float32)
    bmask[0, 0] = bmask[64, 0] = 0.0
    bmask[63, 1] = bmask[127, 1] = 0.0

    return {
        "xz": np.ascontiguousarray(
            xzarr.reshape(2, 128, NWIN)).astype(np.float16),
        "wdef": wdef.astype(np.float16), "woffz": woffz.astype(np.float16),
        "bxy": np.ascontiguousarray(bxy), "bmask": bmask,
        "wconst": wconst,
    }


def kernel(x, offset_w, offset_b, deform_w, gn_gamma, gn_beta):
    x = np.asarray(x, np.float32)
    offw = np.asarray(offset_w, np.float32)
    offb = np.asarray(offset_b, np.float32)
    dw = np.asarray(deform_w, np.float32)
    gamma = np.asarray(gn_gamma, np.float32)
    beta = np.asarray(gn_beta, np.float32)

    nc = _program()

    onescol = np.ones((128, 1), np.float32)
    onesrow = np.ones((1, 128), np.float32)
    gnab = np.concatenate([gamma, beta]).reshape(1, 512).astype(np.float32)
    # pmat[u, q, m] = 1 iff q == 16u + (m % 16)
    pmat = np.zeros((8, 128, 128), np.float32)
    for u in range(8):
        for m in range(128):
            pmat[u, 16 * u + (m % 16), m] = 1.0

    in_maps = []
    for core in range(8):
        m = _prep_core(core, x, offw, offb, dw)
        m.update({"onescol": onescol, "onesrow": onesrow,
                  "gnab": gnab, "pmat": pmat})
        in_maps.append(m)

    global _last_in_maps
    _last_in_maps = in_maps

    res = run_bass_kernel_spmd(nc, in_maps, core_ids=list(range(8)))

    out = np.zeros((B, COUT, H, W), np.float32)
    for core in range(8):
        b, h = core // 2, core % 2
        o = res.results[core]["out"]  # [2048, 256]
        out[b, :, 32 * h:32 * h + 32, :] = (
            o.reshape(32, 64, COUT).transpose(2, 0, 1))
    return out
